# revision 1
# baseline (speedup 1.0000x reference)
"""Trainium2 Bass kernel for nn_DecodePredictions (YOLO-style decode, B=16).

Strategy: pure data-parallel over batch (2 images per core x 8 cores).
Host-side: concat the 3 prediction levels into a flat [N_anchor, 85] tensor
per image, pad 8400 -> 8448 anchors so everything divides evenly, and lay
anchors out partition-blocked so every DMA moves large contiguous
per-partition chunks. The box fields are split host-side into (x,y) and
(w,h) planes so the whole box decode runs once up front on contiguous
layouts (single Exp -> one ACT table load); the per-tile ACT work is then a
single contiguous Sigmoid. Per-anchor grid/stride constants are precomputed
on host (input-independent) and streamed in as tiny aux tensors.

Device-side: box_all = decode(pa01, pa23, aux) once; then per tile of 6
anchors/partition: sigmoid logits, broadcast box into out lanes 0:4 (step-0
AP), scores into lane 5; the constant class-id lane 4 lives in persistent
output buffers written once at init. Output [128, 132, 80, 6] per core is
bit-identical in layout to the final [B, N*C, 6] rows owned by that core.
"""

import ml_dtypes
import numpy as np

N_CORES = 8
B = 16
B_PER_CORE = B // N_CORES  # 2
C = 80
F = 85
N_REAL = 8400              # 80*80 + 40*40 + 20*20
N_PAD = 8448               # = 66 * 128
P = 128
KPP = B_PER_CORE * N_PAD // P  # 132 anchors per partition
GK = 6                     # anchors per partition per tile
NT = KPP // GK             # 22 tiles
NOB = 6                    # persistent output buffers

_CACHE: dict = {}


def _build_nc():
    import concourse.bacc as bacc
    import concourse.tile as tile
    from concourse import mybir
    from contextlib import ExitStack

    nc = bacc.Bacc("TRN2", target_bir_lowering=False, debug=False)
    pa01 = nc.dram_tensor("pa01", [P, KPP, 2], mybir.dt.float32, kind="ExternalInput")
    pa23 = nc.dram_tensor("pa23", [P, KPP, 2], mybir.dt.float32, kind="ExternalInput")
    auxS = nc.dram_tensor("auxS", [P, KPP, 2], mybir.dt.float32, kind="ExternalInput")
    auxB = nc.dram_tensor("auxB", [P, KPP, 2], mybir.dt.float32, kind="ExternalInput")
    predsB = nc.dram_tensor("predsB", [P, KPP, 81], mybir.dt.bfloat16, kind="ExternalInput")
    clsc = nc.dram_tensor("clsc", [P, C], mybir.dt.float32, kind="ExternalInput")
    out = nc.dram_tensor("out", [P, KPP, C, 6], mybir.dt.float32, kind="ExternalOutput")

    fp32 = mybir.dt.float32
    AF = mybir.ActivationFunctionType

    with tile.TileContext(nc) as tc, ExitStack() as ctx:
        cpool = ctx.enter_context(tc.tile_pool(name="const", bufs=1))
        ipool = ctx.enter_context(tc.tile_pool(name="in", bufs=10))
        opool = ctx.enter_context(tc.tile_pool(name="out", bufs=1))
        tpool = ctx.enter_context(tc.tile_pool(name="tmp", bufs=6))

        pa23_t = cpool.tile([P, KPP, 2], fp32, tag="pa23")
        nc.sync.dma_start(out=pa23_t[:], in_=pa23[:])
        auxS_t = cpool.tile([P, KPP, 2], fp32, tag="auxS")
        nc.sync.dma_start(out=auxS_t[:], in_=auxS[:])
        pa01_t = cpool.tile([P, KPP, 2], fp32, tag="pa01")
        nc.sync.dma_start(out=pa01_t[:], in_=pa01[:])
        auxB_t = cpool.tile([P, KPP, 2], fp32, tag="auxB")
        nc.sync.dma_start(out=auxB_t[:], in_=auxB[:])
        cls_t = cpool.tile([P, C], fp32, tag="cls")
        nc.gpsimd.dma_start(out=cls_t[:], in_=clsc[:])

        # Whole-core box decode once, all on contiguous layouts:
        #   bb[:,0] = p_xy * s + grid*s ; bb[:,1] = bb[:,0] + exp(p_wh) * s
        # then interleave into box_t[p, k, 0:4] = (x1, y1, x2, y2).
        wh_t = cpool.tile([P, KPP, 2], fp32, tag="wh")
        nc.scalar.activation(wh_t[:], pa23_t[:], AF.Exp)
        nc.vector.tensor_mul(wh_t[:], wh_t[:], auxS_t[:])
        bb_t = cpool.tile([P, 2, KPP, 2], fp32, tag="bb")
        nc.vector.tensor_mul(bb_t[:, 0, :, :], pa01_t[:], auxS_t[:])
        nc.vector.tensor_add(bb_t[:, 0, :, :], bb_t[:, 0, :, :], auxB_t[:])
        nc.vector.tensor_add(bb_t[:, 1, :, :], bb_t[:, 0, :, :], wh_t[:])
        box_t = cpool.tile([P, KPP, 4], fp32, tag="box")
        nc.vector.tensor_copy(
            box_t[:].rearrange("p k (jh jl) -> p k jh jl", jh=2),
            bb_t[:].rearrange("p jh k jl -> p k jh jl"),
        )

        # Persistent out buffers; constant class-id lane written once (on the
        # Scalar engine, which is otherwise idle, to keep GpSimd off DVE's
        # SBUF ports during the box-decode chain).
        ot_bufs = [
            opool.tile([P, GK, C, 6], fp32, tag=f"ot{j}", name=f"ot{j}")
            for j in range(NOB)
        ]
        for j in range(NOB):
            nc.scalar.copy(
                ot_bufs[j][:, :, :, 4:5],
                cls_t[:].unsqueeze(1).unsqueeze(3).broadcast_to([P, GK, C, 1]),
            )

        pt2 = None
        for t in range(NT):
            sl = slice(t * GK, (t + 1) * GK)
            if t % 2 == 0:
                # One input DMA feeds two compute tiles: per-partition chunks
                # double to 3888 B, halving descriptor + issue overhead.
                pt2 = ipool.tile([P, 2 * GK, 81], mybir.dt.bfloat16, tag="pt", name=f"pt{t}")
                # First few input tiles ride the fast HWDGE queue so the DMA
                # engines have bytes to move during the startup window.
                dma_eng = nc.sync if t < 6 else nc.gpsimd
                dma_eng.dma_start(
                    out=pt2[:], in_=predsB[:, t * GK : (t + 2) * GK, :]
                )

            sig = tpool.tile([P, GK, 81], fp32, tag="sig")
            half = t % 2
            nc.scalar.activation(
                sig[:], pt2[:, half * GK : (half + 1) * GK, :], AF.Sigmoid
            )

            ot = ot_bufs[t % NOB]
            nc.vector.tensor_copy(
                ot[:, :, :, 0:4],
                box_t[:, sl, :].unsqueeze(2).broadcast_to([P, GK, C, 4]),
            )
            nc.vector.tensor_mul(
                ot[:, :, :, 5:6],
                sig[:, :, 1:81].unsqueeze(3),
                sig[:, :, 0:1].broadcast_to([P, GK, C]).unsqueeze(3),
            )

            nc.sync.dma_start(out=out[:, sl, :, :], in_=ot[:])

    nc.compile()
    return nc


def _host_consts():
    # Per-anchor (stride, stride) and (gx*stride, gy*stride), padded to N_PAD.
    s = np.ones(N_PAD, np.float32)
    bx = np.zeros(N_PAD, np.float32)
    by = np.zeros(N_PAD, np.float32)
    off = 0
    for g, st in ((80, 8.0), (40, 16.0), (20, 32.0)):
        n = g * g
        i = np.arange(n)
        s[off : off + n] = st
        bx[off : off + n] = (i % g) * st
        by[off : off + n] = (i // g) * st
        off += n
    auxS = np.stack([s, s], axis=-1).astype(np.float32)
    auxB = np.stack([bx, by], axis=-1).astype(np.float32)
    auxS = np.concatenate([auxS] * B_PER_CORE, 0).reshape(P, KPP, 2)
    auxB = np.concatenate([auxB] * B_PER_CORE, 0).reshape(P, KPP, 2)
    cls = np.broadcast_to(np.arange(C, dtype=np.float32), (P, C)).copy()
    return np.ascontiguousarray(auxS), np.ascontiguousarray(auxB), cls


def _host_in_maps(pred0, pred1, pred2):
    auxS, auxB, cls = _CACHE["consts"]
    pred0 = np.asarray(pred0, np.float32).reshape(B, -1, F)
    pred1 = np.asarray(pred1, np.float32).reshape(B, -1, F)
    pred2 = np.asarray(pred2, np.float32).reshape(B, -1, F)
    in_maps = []
    for core in range(N_CORES):
        flat = np.zeros((B_PER_CORE * N_PAD, F), np.float32)
        for j in range(B_PER_CORE):
            b = core * B_PER_CORE + j
            flat[j * N_PAD : j * N_PAD + N_REAL] = np.concatenate(
                [pred0[b], pred1[b], pred2[b]], axis=0
            )
        in_maps.append(
            {
                "pa01": np.ascontiguousarray(flat[:, 0:2]).reshape(P, KPP, 2),
                "pa23": np.ascontiguousarray(flat[:, 2:4]).reshape(P, KPP, 2),
                "auxS": auxS,
                "auxB": auxB,
                "predsB": np.ascontiguousarray(flat[:, 4:85]).astype(ml_dtypes.bfloat16).reshape(P, KPP, 81),
                "clsc": cls,
            }
        )
    return in_maps


def kernel(images, pred0, pred1, pred2):
    from concourse.bass_utils import run_bass_kernel_spmd

    if "nc" not in _CACHE:
        _CACHE["consts"] = _host_consts()
        _CACHE["nc"] = _build_nc()
    nc = _CACHE["nc"]

    in_maps = _host_in_maps(pred0, pred1, pred2)
    res = run_bass_kernel_spmd(nc, in_maps, list(range(N_CORES)))
    outs = [
        r["out"].reshape(B_PER_CORE, N_PAD * C, 6)[:, : N_REAL * C, :]
        for r in res.results
    ]
    return np.concatenate(outs, axis=0)



# revision 2
# speedup vs baseline: 2.8233x; 2.8233x over previous
"""Trainium2 Bass kernel for nn_DecodePredictions (YOLO-style decode, B=16).

Strategy: pure data-parallel over batch (2 images per core x 8 cores).

The reference output [B, N*C, 6] is 80x redundant: lanes 0:4 (the box) are
broadcast over the 80 classes and lane 4 is the constant class id.  The
device computes only the unique values -- boxes (exp + mul/add chain) and
the 80 per-class scores (sigmoid(obj)*sigmoid(cls)) -- and the host
replicates them into the final layout while unsharding.  That drops
per-core HBM traffic from ~36 MB (fp32 broadcast output) to ~6 MB, which
is what bounds this memory-regime kernel.

Device-side per core (P=128 partitions, KPP=132 anchors/partition):
  in : pa01/pa23 [P,KPP,2] f32 box logits, auxS/auxB [P,KPP,2] f32
       per-anchor (stride, grid*stride) constants, predsB [P,KPP,82] bf16
       score logits laid out [obj, pad, cls*80] so every DVE access is
       32-bit aligned (2x perf mode).
  box: bb[:,0]=pa01*auxS+auxB ; bb[:,1]=bb[:,0]+exp(pa23)*auxS  (planar)
  sco: per tile of 33 anchors/partition: sig=Sigmoid(preds) bf16, then
       one DVE mul with the broadcast obj column -> scores bf16.
  out: bb [P,2,KPP,2] f32 (270KB) + scores [P,KPP,80] bf16 (2.7MB).

Host-side: concat/pad the 3 levels to 8448 anchors, split planes, build
the full [B, N*C, 6] fp32 array from the compact device outputs.
"""

import ml_dtypes
import numpy as np

N_CORES = 8
B = 16
B_PER_CORE = B // N_CORES  # 2
C = 80
F = 85
CH = 82                    # device score-logit channels: [obj, pad, cls*80]
N_REAL = 8400              # 80*80 + 40*40 + 20*20
N_PAD = 8448               # = 66 * 128
P = 128
KPP = B_PER_CORE * N_PAD // P  # 132 anchors per partition
NT = 4                     # score tiles
KT = KPP // NT             # 33 anchors per partition per tile

_CACHE: dict = {}


def _build_nc():
    import concourse.bacc as bacc
    import concourse.tile as tile
    from concourse import mybir
    from contextlib import ExitStack

    nc = bacc.Bacc("TRN2", target_bir_lowering=False, debug=False)
    pa01 = nc.dram_tensor("pa01", [P, KPP, 2], mybir.dt.float32, kind="ExternalInput")
    pa23 = nc.dram_tensor("pa23", [P, KPP, 2], mybir.dt.float32, kind="ExternalInput")
    auxS = nc.dram_tensor("auxS", [P, KPP, 2], mybir.dt.float32, kind="ExternalInput")
    auxB = nc.dram_tensor("auxB", [P, KPP, 2], mybir.dt.float32, kind="ExternalInput")
    predsB = nc.dram_tensor("predsB", [P, KPP, CH], mybir.dt.bfloat16, kind="ExternalInput")
    bb = nc.dram_tensor("bb", [P, 2, KPP, 2], mybir.dt.float32, kind="ExternalOutput")
    scores = nc.dram_tensor("scores", [P, KPP, C], mybir.dt.bfloat16, kind="ExternalOutput")

    fp32 = mybir.dt.float32
    bf16 = mybir.dt.bfloat16
    AF = mybir.ActivationFunctionType

    with tile.TileContext(nc) as tc, ExitStack() as ctx:
        cpool = ctx.enter_context(tc.tile_pool(name="const", bufs=1))
        ipool = ctx.enter_context(tc.tile_pool(name="in", bufs=1))
        spool = ctx.enter_context(tc.tile_pool(name="sig", bufs=2))
        opool = ctx.enter_context(tc.tile_pool(name="sc", bufs=2))

        # Small box inputs first on the HWDGE queue so the ACT exp (and its
        # table load) can start while the preds tiles stream in behind them.
        pa23_t = cpool.tile([P, KPP, 2], fp32, tag="pa23")
        nc.sync.dma_start(out=pa23_t[:], in_=pa23[:])
        auxS_t = cpool.tile([P, KPP, 2], fp32, tag="auxS")
        nc.sync.dma_start(out=auxS_t[:], in_=auxS[:])
        pa01_t = cpool.tile([P, KPP, 2], fp32, tag="pa01")
        nc.sync.dma_start(out=pa01_t[:], in_=pa01[:])
        auxB_t = cpool.tile([P, KPP, 2], fp32, tag="auxB")
        nc.sync.dma_start(out=auxB_t[:], in_=auxB[:])

        pt = []
        for t in range(NT):
            ptile = ipool.tile([P, KT, CH], bf16, tag=f"pt{t}", name=f"pt{t}")
            nc.sync.dma_start(out=ptile[:], in_=predsB[:, t * KT : (t + 1) * KT, :])
            pt.append(ptile)

        # Box decode: planar (x1,y1) and (x2,y2) planes; host interleaves.
        wh_t = cpool.tile([P, KPP, 2], fp32, tag="wh")
        nc.scalar.activation(wh_t[:], pa23_t[:], AF.Exp)
        nc.vector.tensor_mul(wh_t[:], wh_t[:], auxS_t[:])
        bb_t = cpool.tile([P, 2, KPP, 2], fp32, tag="bb")
        nc.vector.tensor_mul(bb_t[:, 0, :, :], pa01_t[:], auxS_t[:])
        nc.vector.tensor_add(bb_t[:, 0, :, :], bb_t[:, 0, :, :], auxB_t[:])
        nc.vector.tensor_add(bb_t[:, 1, :, :], bb_t[:, 0, :, :], wh_t[:])
        nc.gpsimd.dma_start(out=bb[:], in_=bb_t[:])

        for t in range(NT):
            sl = slice(t * KT, (t + 1) * KT)
            sig = spool.tile([P, KT, CH], bf16, tag="sig")
            nc.scalar.activation(sig[:], pt[t][:], AF.Sigmoid)
            sc = opool.tile([P, KT, C], bf16, tag="sc")
            nc.vector.tensor_mul(
                sc[:],
                sig[:, :, 2:CH],
                sig[:, :, 0:1].broadcast_to([P, KT, C]),
            )
            nc.gpsimd.dma_start(out=scores[:, sl, :], in_=sc[:])

    nc.compile()
    return nc


def _host_consts():
    # Per-anchor (stride, stride) and (gx*stride, gy*stride), padded to N_PAD.
    s = np.ones(N_PAD, np.float32)
    bx = np.zeros(N_PAD, np.float32)
    by = np.zeros(N_PAD, np.float32)
    off = 0
    for g, st in ((80, 8.0), (40, 16.0), (20, 32.0)):
        n = g * g
        i = np.arange(n)
        s[off : off + n] = st
        bx[off : off + n] = (i % g) * st
        by[off : off + n] = (i // g) * st
        off += n
    auxS = np.stack([s, s], axis=-1).astype(np.float32)
    auxB = np.stack([bx, by], axis=-1).astype(np.float32)
    auxS = np.concatenate([auxS] * B_PER_CORE, 0).reshape(P, KPP, 2)
    auxB = np.concatenate([auxB] * B_PER_CORE, 0).reshape(P, KPP, 2)
    return np.ascontiguousarray(auxS), np.ascontiguousarray(auxB)


def _host_in_maps(pred0, pred1, pred2):
    auxS, auxB = _CACHE["consts"]
    pred0 = np.asarray(pred0, np.float32).reshape(B, -1, F)
    pred1 = np.asarray(pred1, np.float32).reshape(B, -1, F)
    pred2 = np.asarray(pred2, np.float32).reshape(B, -1, F)
    in_maps = []
    for core in range(N_CORES):
        flat = np.zeros((B_PER_CORE * N_PAD, F), np.float32)
        for j in range(B_PER_CORE):
            b = core * B_PER_CORE + j
            flat[j * N_PAD : j * N_PAD + N_REAL] = np.concatenate(
                [pred0[b], pred1[b], pred2[b]], axis=0
            )
        logits = np.zeros((B_PER_CORE * N_PAD, CH), np.float32)
        logits[:, 0] = flat[:, 4]
        logits[:, 2:CH] = flat[:, 5:F]
        in_maps.append(
            {
                "pa01": np.ascontiguousarray(flat[:, 0:2]).reshape(P, KPP, 2),
                "pa23": np.ascontiguousarray(flat[:, 2:4]).reshape(P, KPP, 2),
                "auxS": auxS,
                "auxB": auxB,
                "predsB": logits.astype(ml_dtypes.bfloat16).reshape(P, KPP, CH),
            }
        )
    return in_maps


def kernel(images, pred0, pred1, pred2):
    from concourse.bass_utils import run_bass_kernel_spmd

    if "nc" not in _CACHE:
        _CACHE["consts"] = _host_consts()
        _CACHE["nc"] = _build_nc()
    nc = _CACHE["nc"]

    in_maps = _host_in_maps(pred0, pred1, pred2)
    res = run_bass_kernel_spmd(nc, in_maps, list(range(N_CORES)))

    full = np.empty((B, N_REAL, C, 6), np.float32)
    full[:, :, :, 4] = np.arange(C, dtype=np.float32)
    for core, r in enumerate(res.results):
        b0 = core * B_PER_CORE
        # bb [P,2,KPP,2] -> [img, anchor, (x1,y1,x2,y2)]
        boxes = (
            np.asarray(r["bb"])
            .transpose(0, 2, 1, 3)
            .reshape(B_PER_CORE, N_PAD, 4)[:, :N_REAL]
        )
        sc = (
            np.asarray(r["scores"])
            .astype(np.float32)
            .reshape(B_PER_CORE, N_PAD, C)[:, :N_REAL]
        )
        full[b0 : b0 + B_PER_CORE, :, :, 0:4] = boxes[:, :, None, :]
        full[b0 : b0 + B_PER_CORE, :, :, 5] = sc
    return full.reshape(B, N_REAL * C, 6)


# revision 3
# speedup vs baseline: 3.4670x; 1.2280x over previous
"""Trainium2 Bass kernel for nn_DecodePredictions (YOLO-style decode, B=16).

Strategy: pure data-parallel over batch (2 images per core x 8 cores).

The reference output [B, N*C, 6] is 80x redundant: lanes 0:4 (the box) are
broadcast over the 80 classes and lane 4 is the constant class id.  The
device computes only the unique values -- boxes (exp + mul/add chain) and
the 80 per-class scores (sigmoid(obj)*sigmoid(cls)) -- and the host
replicates them into the final layout while unsharding.  That drops
per-core HBM traffic from ~36 MB (fp32 broadcast output) to ~3 MB.

Per core (P=128 partitions, KPP=132 anchors/partition, 2 images):
  in : pab  [P,KPP,4] fp16  box logits (px,py,pw,ph)
       auxp [P,KPP,4] fp16  per-anchor (s, s, gx*s, gy*s) constants
       predsB [P,81*KPP] fp8e3  score logits in channel-major tile blocks
       [81, KT] so the DVE multiply sees stride-1 inner dims on all
       operands (2x perf mode; the obj broadcast rides the outer dim).
  sco: per tile: sig = Sigmoid(preds) bf16 (ACT), one DVE mul with the
       broadcast obj row -> bf16, cast to fp8e3 by the SWDGE out-DMA.
  box: epilogue after the sigmoids (one ACT table switch): wh=Exp(pwh)*s,
       bb[:,0]=pxy*s+grid*s, bb[:,1]=bb[:,0]+wh, planar fp16 out.
  out: bb [P,2,KPP,2] fp16 (135KB) + scores [P,80*KPP] fp8e3 (1.35MB).

Tiles are [12,40,40,40] anchors/partition: a small first tile so the ACT
engine starts ~1us after the first DMA lands, big tiles after to amortize
the 352-cycle ACTIVATE overhead.

Host-side: concat/pad the 3 levels to 8448 anchors, pack the per-tile
channel-major fp8 blocks, and assemble the full [B, N*C, 6] fp32 output
from the compact device outputs.
"""

import ml_dtypes
import numpy as np

N_CORES = 8
B = 16
B_PER_CORE = B // N_CORES  # 2
C = 80
F = 85
CH = 81                    # obj + 80 cls
N_REAL = 8400              # 80*80 + 40*40 + 20*20
N_PAD = 8448               # = 66 * 128
P = 128
KPP = B_PER_CORE * N_PAD // P  # 132 anchors per partition
KTS = (12, 40, 40, 40)     # score-tile sizes (anchors/partition)
OFFS = (0, 12, 52, 92)

_CACHE: dict = {}


def _build_nc():
    import concourse.bacc as bacc
    import concourse.tile as tile
    from concourse import mybir
    from contextlib import ExitStack

    nc = bacc.Bacc("TRN2", target_bir_lowering=False, debug=False)
    pab = nc.dram_tensor("pab", [P, KPP, 4], mybir.dt.float16, kind="ExternalInput")
    auxp = nc.dram_tensor("auxp", [P, KPP, 4], mybir.dt.float16, kind="ExternalInput")
    predsB = nc.dram_tensor("predsB", [P, CH * KPP], mybir.dt.float8e3, kind="ExternalInput")
    bb = nc.dram_tensor("bb", [P, 2, KPP, 2], mybir.dt.float16, kind="ExternalOutput")
    scores = nc.dram_tensor("scores", [P, C * KPP], mybir.dt.float8e3, kind="ExternalOutput")

    fp16 = mybir.dt.float16
    bf16 = mybir.dt.bfloat16
    fp8 = mybir.dt.float8e3
    AF = mybir.ActivationFunctionType

    with tile.TileContext(nc) as tc, ExitStack() as ctx:
        cpool = ctx.enter_context(tc.tile_pool(name="const", bufs=1))
        spool = ctx.enter_context(tc.tile_pool(name="sig", bufs=2))
        opool = ctx.enter_context(tc.tile_pool(name="sc", bufs=2))

        # Preds tiles first on the HWDGE queue: the first sigmoid gates
        # everything downstream, so its (small) tile leads.
        pt = []
        for t, kt in enumerate(KTS):
            ptile = cpool.tile([P, CH, kt], fp8, tag=f"pt{t}", name=f"pt{t}")
            nc.sync.dma_start(
                out=ptile[:], in_=predsB[:, CH * OFFS[t] : CH * (OFFS[t] + kt)]
            )
            pt.append(ptile)
        pab_t = cpool.tile([P, KPP, 4], fp16, tag="pab")
        nc.sync.dma_start(out=pab_t[:], in_=pab[:])
        auxp_t = cpool.tile([P, KPP, 4], fp16, tag="auxp")
        nc.sync.dma_start(out=auxp_t[:], in_=auxp[:])

        for t, kt in enumerate(KTS):
            sig = spool.tile([P, CH, kt], bf16, tag=f"sig{t % 2}", name=f"sig{t}")
            nc.scalar.activation(sig[:], pt[t][:], AF.Sigmoid)
            sc = opool.tile([P, C, kt], bf16, tag=f"sc{t % 2}", name=f"sc{t}")
            nc.vector.tensor_mul(
                sc[:],
                sig[:, 1:CH, :],
                sig[:, 0:1, :].broadcast_to([P, C, kt]),
            )
            # SWDGE casts bf16 -> fp8e3 on the way to HBM.
            nc.gpsimd.dma_start(
                out=scores[:, C * OFFS[t] : C * (OFFS[t] + kt)], in_=sc[:]
            )

        # Box epilogue: one ACT table switch to Exp after the sigmoids.
        wh_t = cpool.tile([P, KPP, 2], fp16, tag="wh")
        nc.scalar.activation(wh_t[:], pab_t[:, :, 2:4], AF.Exp)
        bb_t = cpool.tile([P, 2, KPP, 2], fp16, tag="bb")
        nc.vector.tensor_mul(wh_t[:], wh_t[:], auxp_t[:, :, 0:2])
        nc.vector.tensor_mul(bb_t[:, 0, :, :], pab_t[:, :, 0:2], auxp_t[:, :, 0:2])
        nc.vector.tensor_add(bb_t[:, 0, :, :], bb_t[:, 0, :, :], auxp_t[:, :, 2:4])
        nc.vector.tensor_add(bb_t[:, 1, :, :], bb_t[:, 0, :, :], wh_t[:])
        nc.gpsimd.dma_start(out=bb[:], in_=bb_t[:])

    nc.compile()
    return nc


def _host_consts():
    # Per-anchor (stride, stride, gx*stride, gy*stride), padded to N_PAD.
    s = np.ones(N_PAD, np.float32)
    bx = np.zeros(N_PAD, np.float32)
    by = np.zeros(N_PAD, np.float32)
    off = 0
    for g, st in ((80, 8.0), (40, 16.0), (20, 32.0)):
        n = g * g
        i = np.arange(n)
        s[off : off + n] = st
        bx[off : off + n] = (i % g) * st
        by[off : off + n] = (i // g) * st
        off += n
    auxp = np.stack([s, s, bx, by], axis=-1).astype(np.float16)
    auxp = np.concatenate([auxp] * B_PER_CORE, 0).reshape(P, KPP, 4)
    return np.ascontiguousarray(auxp)


def _host_in_maps(pred0, pred1, pred2):
    auxp = _CACHE["consts"]
    pred0 = np.asarray(pred0, np.float32).reshape(B, -1, F)
    pred1 = np.asarray(pred1, np.float32).reshape(B, -1, F)
    pred2 = np.asarray(pred2, np.float32).reshape(B, -1, F)
    in_maps = []
    for core in range(N_CORES):
        flat = np.zeros((B_PER_CORE * N_PAD, F), np.float32)
        for j in range(B_PER_CORE):
            b = core * B_PER_CORE + j
            flat[j * N_PAD : j * N_PAD + N_REAL] = np.concatenate(
                [pred0[b], pred1[b], pred2[b]], axis=0
            )
        # Channel-major per tile: block t is [CH, KT_t] per partition.
        lg = (
            flat[:, 4:F]
            .astype(ml_dtypes.float8_e3m4)
            .reshape(P, KPP, CH)
        )
        blocks = [
            np.ascontiguousarray(lg[:, OFFS[t] : OFFS[t] + kt, :].transpose(0, 2, 1))
            for t, kt in enumerate(KTS)
        ]
        predsB = np.concatenate([b.reshape(P, -1) for b in blocks], axis=1)
        in_maps.append(
            {
                "pab": np.ascontiguousarray(flat[:, 0:4]).astype(np.float16).reshape(P, KPP, 4),
                "auxp": auxp,
                "predsB": np.ascontiguousarray(predsB),
            }
        )
    return in_maps


def kernel(images, pred0, pred1, pred2):
    from concourse.bass_utils import run_bass_kernel_spmd

    if "nc" not in _CACHE:
        _CACHE["consts"] = _host_consts()
        _CACHE["nc"] = _build_nc()
    nc = _CACHE["nc"]

    in_maps = _host_in_maps(pred0, pred1, pred2)
    res = run_bass_kernel_spmd(nc, in_maps, list(range(N_CORES)))

    full = np.empty((B, N_REAL, C, 6), np.float32)
    full[:, :, :, 4] = np.arange(C, dtype=np.float32)
    for core, r in enumerate(res.results):
        b0 = core * B_PER_CORE
        boxes = (
            np.asarray(r["bb"])
            .astype(np.float32)
            .transpose(0, 2, 1, 3)
            .reshape(B_PER_CORE, N_PAD, 4)[:, :N_REAL]
        )
        sc_flat = np.asarray(r["scores"])  # [P, C*KPP] fp8e3 in tile blocks
        parts = []
        for t, kt in enumerate(KTS):
            blk = sc_flat[:, C * OFFS[t] : C * (OFFS[t] + kt)].reshape(P, C, kt)
            parts.append(blk.transpose(0, 2, 1))  # [P, kt, C]
        sc = (
            np.concatenate(parts, axis=1)
            .astype(np.float32)
            .reshape(B_PER_CORE, N_PAD, C)[:, :N_REAL]
        )
        full[b0 : b0 + B_PER_CORE, :, :, 0:4] = boxes[:, :, None, :]
        full[b0 : b0 + B_PER_CORE, :, :, 5] = sc
    return full.reshape(B, N_REAL * C, 6)


# revision 8
# speedup vs baseline: 3.5832x; 1.0335x over previous
"""Trainium2 Bass kernel for nn_DecodePredictions (YOLO-style decode, B=16).

Strategy: pure data-parallel over batch (2 images per core x 8 cores).

The reference output [B, N*C, 6] is 80x redundant: lanes 0:4 (the box) are
broadcast over the 80 classes and lane 4 is the constant class id.  The
device computes only the unique values -- boxes (exp + mul/add chain) and
the 80 per-class scores (sigmoid(obj)*sigmoid(cls)) -- and the host
replicates them into the final layout while unsharding.  That drops
per-core HBM traffic from ~36 MB (fp32 broadcast output) to ~3 MB.

Per core (P=128 partitions, KPP=132 anchors/partition, 2 images):
  in : pab  [P,KPP,4] fp16  box logits (px,py,pw,ph)
       auxp [P,KPP,4] fp16  per-anchor (s, s, gx*s, gy*s) constants
       predsB [P,81*KPP] fp8e3  score logits in channel-major tile blocks
       [81, KT] so the DVE multiply sees stride-1 inner dims on all
       operands (2x perf mode; the obj broadcast rides the outer dim).
  sco: per tile: sig = Sigmoid(preds) bf16 (ACT), one DVE mul with the
       broadcast obj row -> bf16 scores out.  (A cast-to-fp8 out-DMA was
       tried and reverted: the SDMA cast path is SBUF-read-bound, so it
       moves no faster than writing bf16 directly.)
  box: epilogue after the sigmoids (one ACT table switch): wh=Exp(pwh)*s,
       bb[:,0]=pxy*s+grid*s, bb[:,1]=bb[:,0]+wh, planar fp16 out.
  out: bb [P,2,KPP,2] fp16 (135KB) + scores [P,80*KPP] bf16 (2.7MB).

Tiles are [20,44,36,32] anchors/partition, sized so each tile's DMA lands
just before the ACT engine finishes the previous sigmoid (no gaps).  The
last tile's multiply + store are split in half to shorten the tail, and
the final stores ride the HWDGE ring whose completion latency (~0.5us) is
3x lower than SWDGE's; earlier stores go on the parallel SWDGE queue.

Host-side: concat/pad the 3 levels to 8448 anchors, pack the per-tile
channel-major fp8 blocks, and assemble the full [B, N*C, 6] fp32 output
from the compact device outputs.
"""

import ml_dtypes
import numpy as np

N_CORES = 8
B = 16
B_PER_CORE = B // N_CORES  # 2
C = 80
F = 85
CH = 81                    # obj + 80 cls
N_REAL = 8400              # 80*80 + 40*40 + 20*20
N_PAD = 8448               # = 66 * 128
P = 128
KPP = B_PER_CORE * N_PAD // P  # 132 anchors per partition
KTS = (20, 44, 36, 32)     # score-tile sizes (anchors/partition)
OFFS = (0, 20, 64, 100)

_CACHE: dict = {}


def _build_nc():
    import concourse.bacc as bacc
    import concourse.tile as tile
    from concourse import mybir
    from contextlib import ExitStack

    nc = bacc.Bacc("TRN2", target_bir_lowering=False, debug=False)
    pab = nc.dram_tensor("pab", [P, KPP, 4], mybir.dt.float16, kind="ExternalInput")
    auxp = nc.dram_tensor("auxp", [P, KPP, 4], mybir.dt.float16, kind="ExternalInput")
    predsB = nc.dram_tensor("predsB", [P, CH * KPP], mybir.dt.float8e3, kind="ExternalInput")
    bb = nc.dram_tensor("bb", [P, 2, KPP, 2], mybir.dt.float16, kind="ExternalOutput")
    scores = nc.dram_tensor("scores", [P, C * KPP], mybir.dt.bfloat16, kind="ExternalOutput")

    fp16 = mybir.dt.float16
    bf16 = mybir.dt.bfloat16
    fp8 = mybir.dt.float8e3
    AF = mybir.ActivationFunctionType

    with tile.TileContext(nc) as tc, ExitStack() as ctx:
        cpool = ctx.enter_context(tc.tile_pool(name="const", bufs=1))
        spool = ctx.enter_context(tc.tile_pool(name="sig", bufs=2))
        opool = ctx.enter_context(tc.tile_pool(name="sc", bufs=2))

        # Preds tiles first on the HWDGE queue: the first sigmoid gates
        # everything downstream, so its (small) tile leads.
        pt = []
        for t, kt in enumerate(KTS):
            ptile = cpool.tile([P, CH, kt], fp8, tag=f"pt{t}", name=f"pt{t}")
            nc.sync.dma_start(
                out=ptile[:], in_=predsB[:, CH * OFFS[t] : CH * (OFFS[t] + kt)]
            )
            pt.append(ptile)
        pab_t = cpool.tile([P, KPP, 4], fp16, tag="pab")
        nc.sync.dma_start(out=pab_t[:], in_=pab[:])
        auxp_t = cpool.tile([P, KPP, 4], fp16, tag="auxp")
        nc.sync.dma_start(out=auxp_t[:], in_=auxp[:])

        for t, kt in enumerate(KTS[:-1]):
            sig = spool.tile([P, CH, kt], bf16, tag=f"sig{t % 2}", name=f"sig{t}")
            nc.scalar.activation(sig[:], pt[t][:], AF.Sigmoid)
            sc = opool.tile([P, C, kt], bf16, tag=f"sc{t % 2}", name=f"sc{t}")
            nc.vector.tensor_mul(
                sc[:],
                sig[:, 1:CH, :],
                sig[:, 0:1, :].broadcast_to([P, C, kt]),
            )
            nc.gpsimd.dma_start(
                out=scores[:, C * OFFS[t] : C * (OFFS[t] + kt)], in_=sc[:]
            )

        # Last tile: multiply + store in two halves on the fast-receipt
        # HWDGE ring so the tail is two short dependency chains.
        t3, kt3 = len(KTS) - 1, KTS[-1]
        kh = kt3 // 2
        sig3 = spool.tile([P, CH, kt3], bf16, tag="sig1", name="sig3")
        nc.scalar.activation(sig3[:], pt[t3][:], AF.Sigmoid)
        for h in range(2):
            hs = slice(h * kh, (h + 1) * kh)
            sch = opool.tile([P, C, kh], bf16, tag=f"sch{h}", name=f"sch{h}")
            nc.vector.tensor_mul(
                sch[:],
                sig3[:, 1:CH, hs],
                sig3[:, 0:1, hs].broadcast_to([P, C, kh]),
            )
            o0 = C * (OFFS[t3] + h * kh)
            nc.sync.dma_start(out=scores[:, o0 : o0 + C * kh], in_=sch[:])

        # Box epilogue: one ACT table switch to Exp after the sigmoids.
        wh_t = cpool.tile([P, KPP, 2], fp16, tag="wh")
        nc.scalar.activation(wh_t[:], pab_t[:, :, 2:4], AF.Exp)
        bb_t = cpool.tile([P, 2, KPP, 2], fp16, tag="bb")
        nc.vector.tensor_mul(wh_t[:], wh_t[:], auxp_t[:, :, 0:2])
        nc.vector.tensor_mul(bb_t[:, 0, :, :], pab_t[:, :, 0:2], auxp_t[:, :, 0:2])
        nc.vector.tensor_add(bb_t[:, 0, :, :], bb_t[:, 0, :, :], auxp_t[:, :, 2:4])
        nc.vector.tensor_add(bb_t[:, 1, :, :], bb_t[:, 0, :, :], wh_t[:])
        nc.sync.dma_start(out=bb[:], in_=bb_t[:])

    nc.compile()
    return nc


def _host_consts():
    # Per-anchor (stride, stride, gx*stride, gy*stride), padded to N_PAD.
    s = np.ones(N_PAD, np.float32)
    bx = np.zeros(N_PAD, np.float32)
    by = np.zeros(N_PAD, np.float32)
    off = 0
    for g, st in ((80, 8.0), (40, 16.0), (20, 32.0)):
        n = g * g
        i = np.arange(n)
        s[off : off + n] = st
        bx[off : off + n] = (i % g) * st
        by[off : off + n] = (i // g) * st
        off += n
    auxp = np.stack([s, s, bx, by], axis=-1).astype(np.float16)
    auxp = np.concatenate([auxp] * B_PER_CORE, 0).reshape(P, KPP, 4)
    return np.ascontiguousarray(auxp)


def _host_in_maps(pred0, pred1, pred2):
    auxp = _CACHE["consts"]
    pred0 = np.asarray(pred0, np.float32).reshape(B, -1, F)
    pred1 = np.asarray(pred1, np.float32).reshape(B, -1, F)
    pred2 = np.asarray(pred2, np.float32).reshape(B, -1, F)
    in_maps = []
    for core in range(N_CORES):
        flat = np.zeros((B_PER_CORE * N_PAD, F), np.float32)
        for j in range(B_PER_CORE):
            b = core * B_PER_CORE + j
            flat[j * N_PAD : j * N_PAD + N_REAL] = np.concatenate(
                [pred0[b], pred1[b], pred2[b]], axis=0
            )
        # Channel-major per tile: block t is [CH, KT_t] per partition.
        lg = (
            flat[:, 4:F]
            .astype(ml_dtypes.float8_e3m4)
            .reshape(P, KPP, CH)
        )
        blocks = [
            np.ascontiguousarray(lg[:, OFFS[t] : OFFS[t] + kt, :].transpose(0, 2, 1))
            for t, kt in enumerate(KTS)
        ]
        predsB = np.concatenate([b.reshape(P, -1) for b in blocks], axis=1)
        in_maps.append(
            {
                "pab": np.ascontiguousarray(flat[:, 0:4]).astype(np.float16).reshape(P, KPP, 4),
                "auxp": auxp,
                "predsB": np.ascontiguousarray(predsB),
            }
        )
    return in_maps


def kernel(images, pred0, pred1, pred2):
    from concourse.bass_utils import run_bass_kernel_spmd

    if "nc" not in _CACHE:
        _CACHE["consts"] = _host_consts()
        _CACHE["nc"] = _build_nc()
    nc = _CACHE["nc"]

    in_maps = _host_in_maps(pred0, pred1, pred2)
    res = run_bass_kernel_spmd(nc, in_maps, list(range(N_CORES)))

    full = np.empty((B, N_REAL, C, 6), np.float32)
    full[:, :, :, 4] = np.arange(C, dtype=np.float32)
    for core, r in enumerate(res.results):
        b0 = core * B_PER_CORE
        boxes = (
            np.asarray(r["bb"])
            .astype(np.float32)
            .transpose(0, 2, 1, 3)
            .reshape(B_PER_CORE, N_PAD, 4)[:, :N_REAL]
        )
        sc_flat = np.asarray(r["scores"])  # [P, C*KPP] fp8e3 in tile blocks
        parts = []
        for t, kt in enumerate(KTS):
            blk = sc_flat[:, C * OFFS[t] : C * (OFFS[t] + kt)].reshape(P, C, kt)
            parts.append(blk.transpose(0, 2, 1))  # [P, kt, C]
        sc = (
            np.concatenate(parts, axis=1)
            .astype(np.float32)
            .reshape(B_PER_CORE, N_PAD, C)[:, :N_REAL]
        )
        full[b0 : b0 + B_PER_CORE, :, :, 0:4] = boxes[:, :, None, :]
        full[b0 : b0 + B_PER_CORE, :, :, 5] = sc
    return full.reshape(B, N_REAL * C, 6)
